# revision 30
# baseline (speedup 1.0000x reference)
"""Deformable Conv2d (B=8, C=O=64, H=W=128, K=3) on 8 Trainium2 NeuronCores.

Data-parallel over batch: core b handles batch b.

Per-core algorithm (all on device):
  1. Pad x by 2 (zeros) -> x_pad2 [132,132] so every clamped bilinear corner
     reads an exact zero (reference's out-of-bounds corners contribute 0).
  2. Build P2 in DRAM: token t=(y*132+x) holds [x_pad2[y,x,:64c], x_pad2[y+1,x,:64c]]
     in bf16 (256B). A 512B gather elem starting at token t covers the full
     2x2 corner patch (x and x+1 columns, y and y+1 rows) via elem_step=128.
  3. Pixel-major index/weight math on DVE: y0=floor via round-to-nearest
     magic (the -0.5 is baked into the bx/by base tables), clamp to
     [-1,129], idx=(y0+1)*132+(x0+1); 4 bilinear corner weights.
  4. dma_gather (px-major): multiply by corner weights (pair-packed
     broadcast b operand, bf16 2x mode); 2 adds reduce the dx corner pairs
     -> S_k [128px, j, dy, c]. The dy-sum is absorbed into the conv matmul
     by duplicating weight rows (K-block k = (dy,c), W rows repeated for
     both dy). Gathers are spread round-robin over 4 SWDGE queues so 4 Q7
     core pairs generate descriptors in parallel.
  5. Per kp: PE-transpose S_k to [(dy,c), px]; 9 accumulating matmuls
     -> out[o, px]; ACT bias.
"""

import numpy as np
import ml_dtypes

C = 64
O = 64
H = 128
W = 128
KP = 9
PX = H * W                    # 16384
W2 = 132                      # padded-by-2 width
NTOK = W2 * W2                # 17424
NB = 138                      # x_sb col blocks (138*128 = 17664 >= NTOK+132+...)
NELEM = NTOK - 1              # gather index bound (reads tokens idx, idx+1)
MAGIC = 12582912.0            # 3 * 2**22, f32 round-to-nearest magic
CHUNK = 2048                  # px per gather chunk
NCHUNK = PX // CHUNK          # 8
NJC = CHUNK // 128            # 16 j-blocks per chunk
NJ = PX // 128                # 128 j-blocks total
NSUB = CHUNK // 512           # 4 psum output tiles per chunk
N_CORES = 8
NQ = 4                        # SWDGE queues (Q7 core pairs) for gathers

bf16 = ml_dtypes.bfloat16

_CACHE = {}


def _build_program(repeat=1):
    import concourse.bacc as bacc
    import concourse.bass as bass
    import concourse.mybir as mybir
    import concourse.tile as tile
    from concourse import library_config
    from concourse.masks import make_identity

    f32 = mybir.dt.float32
    bff = mybir.dt.bfloat16
    i16 = mybir.dt.int16
    i32 = mybir.dt.int32
    AF = mybir.ActivationFunctionType
    OP = mybir.AluOpType

    nc = bacc.Bacc(
        "TRN2", target_bir_lowering=False, debug=False, num_swdge_queues=NQ,
        dynamic_dma_scratch_size=32768,
    )

    xin = nc.dram_tensor("xin", [C, PX], f32, kind="ExternalInput")
    offin = nc.dram_tensor("offin", [2 * KP, PX], f32, kind="ExternalInput")
    wT = nc.dram_tensor("wT", [KP, 128, O], bff, kind="ExternalInput")
    bin_ = nc.dram_tensor("bin", [O, 1], f32, kind="ExternalInput")
    out = nc.dram_tensor("out", [O, PX], f32, kind="ExternalOutput")
    P2 = nc.dram_tensor("P2", [NTOK * 128], bff)

    with tile.TileContext(nc) as tc:
        with (
            tc.tile_pool(name="const", bufs=1) as cpool,
            tc.tile_pool(name="main", bufs=1) as mpool,
            tc.tile_pool(name="wtmp", bufs=2) as wpool,
            tc.tile_pool(name="psA", bufs=2, space="PSUM") as psA,
            tc.tile_pool(name="psB", bufs=1, space="PSUM") as psB,
        ):
            nc.gpsimd.load_library(library_config.mlp)
            # ---------------- constants ----------------
            ident_bf = cpool.tile([128, 128], bff)
            make_identity(nc, ident_bf[:])
            ident_f = cpool.tile([128, 128], f32)
            make_identity(nc, ident_f[:])

            wT_sb = cpool.tile([128, KP * O], bff)
            nc.sync.dma_start(
                wT_sb[:],
                bass.AP(wT, 0, [[O, 128], [128 * O, KP], [1, O]]),
            )
            bias_sb = cpool.tile([O, 1], f32)
            nc.sync.dma_start(bias_sb[:], bin_.ap())
            c133 = cpool.tile([128, 1], f32, tag="c133")
            nc.vector.memset(c133[:], 133.0)
            c1 = cpool.tile([128, 1], f32, tag="c1")
            nc.vector.memset(c1[:], 1.0)

            # basex (per-partition w) variants: w + kx - 0.5 (the -0.5 turns
            # the magic round-to-nearest into floor; the frac weights add the
            # 0.5 back exactly via scalar_tensor_tensor).
            bx_i = cpool.tile([128, 1], i32)
            nc.gpsimd.iota(bx_i[:], pattern=[[1, 1]], base=0, channel_multiplier=1)
            bx_f = cpool.tile([128, 1], f32)
            nc.vector.tensor_copy(out=bx_f[:], in_=bx_i[:])
            bx = []
            for kxi in range(3):
                t = cpool.tile([128, 1], f32, tag=f"bx{kxi}")
                nc.vector.tensor_scalar(
                    out=t[:], in0=bx_f[:], scalar1=float(kxi) - 0.5, scalar2=None,
                    op0=OP.add,
                )
                bx.append(t)
            # basey (per-column j=h) variants: j + ky - 0.5
            by_i = cpool.tile([128, 128], i32)
            nc.gpsimd.iota(by_i[:], pattern=[[1, 128]], base=0, channel_multiplier=0)
            by_f = cpool.tile([128, 128], f32)
            nc.vector.tensor_copy(out=by_f[:], in_=by_i[:])
            by = []
            for kyi in range(3):
                t = cpool.tile([128, 128], f32, tag=f"by{kyi}")
                nc.vector.tensor_scalar(
                    out=t[:], in0=by_f[:], scalar1=float(kyi) - 0.5, scalar2=None,
                    op0=OP.add,
                )
                by.append(t)

            # ---------------- x -> P2 ----------------
            offp_cm = tc.tile_pool(name="offp", bufs=1)
            opool = offp_cm.__enter__()
            off_px = opool.tile([128, 2 * KP, NJ], f32, tag="offpx")
            # off_f load issued first on sync: it gates the idx/weight chain.
            off_f = opool.tile([2 * KP, PX], f32, tag="offf")
            nc.sync.dma_start(off_f[:], offin.ap())
            with tc.tile_pool(name="prep", bufs=1) as ppool:
                x_sb = ppool.tile([C, NB * 128], bff, tag="xsb")
                # zero only the padding border, not the whole 2.2 MB tile.
                xv = x_sb[:, :NTOK].rearrange("c (a b) -> c a b", a=W2)
                nc.vector.memset(x_sb[:, 0:2 * W2], 0.0)
                nc.vector.memset(x_sb[:, 130 * W2:NB * 128], 0.0)
                nc.vector.memset(xv[:, 2:130, 0:2], 0.0)
                nc.vector.memset(xv[:, 2:130, 130:132], 0.0)
                interior = xv[:, 2:130, 2:130]
                nc.gpsimd.dma_start(
                    out=interior, in_=xin.ap().rearrange("c (h w) -> c h w", h=H)
                )
                x_px = ppool.tile([128, NB, C], bff, tag="xpx")
                nc.sync.dma_start_transpose(x_px[:], x_sb[:])

                # P2 token t first half <- x_tok(t):
                nc.sync.dma_start(
                    bass.AP(P2, 0, [[128, 128], [128 * 128, 136], [1, 64]]),
                    x_px[:, 0:136, :],
                )
                nc.sync.dma_start(
                    bass.AP(P2, 17408 * 128, [[128, 16], [1, 64]]),
                    x_px[0:16, 136, :],
                )
                # P2 token t second half <- x_tok(t+132), written with the
                # -132-token shift on the DRAM side so the SBUF source stays
                # 128-partition aligned (fast descriptors):
                #   t = p + 128b - 132 for src block b>=2  -> t >= 124
                nc.sync.dma_start(
                    bass.AP(P2, 64 + 124 * 128, [[128, 128], [128 * 128, 135], [1, 64]]),
                    x_px[:, 2:137, :],
                )
                #   src block b=1, p>=4  -> t = 0..123
                nc.sync.dma_start(
                    bass.AP(P2, 64, [[128, 124], [1, 64]]),
                    x_px[4:128, 1, :],
                )

                # ---------------- offsets -> px-major ----------------
                for g in range(8):
                    ps = psA.tile([128, 16 * 18], f32, tag="offps")
                    for jj in range(16):
                        j = g * 16 + jj
                        nc.tensor.transpose(
                            out=ps[:, jj * 18:(jj + 1) * 18],
                            in_=off_f[:, j * 128:(j + 1) * 128],
                            identity=ident_f[0:18, 0:18],
                        )
                    nc.vector.tensor_copy(
                        out=off_px[:, :, g * 16:(g + 1) * 16],
                        in_=ps[:].rearrange("p (a b) -> p b a", a=16),
                    )

            # ---------------- per-kp index math ----------------
            idx_all = mpool.tile([128, KP, NJ], i16, tag="idxall")
            frac = mpool.tile([128, 2 * KP, NJ], f32, tag="frac")
            for k in range(KP):
                kyi, kxi = k // 3, k % 3
                oy = off_px[:, 2 * k, :]
                ox = off_px[:, 2 * k + 1, :]
                zy = wpool.tile([128, NJ], f32, tag="zy")
                nc.vector.tensor_tensor(out=zy[:], in0=oy, in1=by[kyi][:], op=OP.add)
                y0 = wpool.tile([128, NJ], f32, tag="y0")
                nc.vector.tensor_scalar(
                    out=y0[:], in0=zy[:], scalar1=MAGIC, scalar2=MAGIC,
                    op0=OP.add, op1=OP.subtract,
                )
                fy = frac[:, 2 * k, :]
                nc.vector.scalar_tensor_tensor(
                    out=fy, in0=zy[:], scalar=0.5, in1=y0[:],
                    op0=OP.add, op1=OP.subtract,
                )
                nc.vector.tensor_scalar(
                    out=y0[:], in0=y0[:], scalar1=-1.0, scalar2=129.0,
                    op0=OP.max, op1=OP.min,
                )
                iy = wpool.tile([128, NJ], f32, tag="iy")
                nc.scalar.activation(
                    out=iy[:], in_=y0[:], func=AF.Identity,
                    bias=c133[:], scale=132.0,
                )

                zx = wpool.tile([128, NJ], f32, tag="zx")
                nc.scalar.activation(
                    out=zx[:], in_=ox, func=AF.Identity,
                    bias=bx[kxi][:], scale=1.0,
                )
                x0 = wpool.tile([128, NJ], f32, tag="x0")
                nc.vector.tensor_scalar(
                    out=x0[:], in0=zx[:], scalar1=MAGIC, scalar2=MAGIC,
                    op0=OP.add, op1=OP.subtract,
                )
                fx = frac[:, 2 * k + 1, :]
                nc.vector.scalar_tensor_tensor(
                    out=fx, in0=zx[:], scalar=0.5, in1=x0[:],
                    op0=OP.add, op1=OP.subtract,
                )
                nc.vector.tensor_scalar(
                    out=x0[:], in0=x0[:], scalar1=-1.0, scalar2=129.0,
                    op0=OP.max, op1=OP.min,
                )
                idxf = wpool.tile([128, NJ], f32, tag="idxf")
                nc.vector.tensor_tensor(out=idxf[:], in0=iy[:], in1=x0[:], op=OP.add)
                nc.vector.tensor_copy(out=idx_all[:, k, :], in_=idxf[:])
            offp_cm.__exit__(None, None, None)

            # ---------------- idx wrap to gather layout ----------------
            # Want: idx_wr[r, k, 8j+s] = idx_all[16s+r, k, j]  (r<16),
            # then replicated to partition groups 16..127. Done in two
            # k-halves so the first gathers start before the whole wrap
            # is finished. Fold/replica DMAs go on the scalar HWDGE queue.
            idx_wr = mpool.tile([128, KP, PX // 16], i16, tag="idxwr")
            with tc.tile_pool(name="wrap", bufs=1) as wrpool:
                tmp16 = wrpool.tile([16, KP, 8, NJ], i16, tag="idxtmp")
                for k0, k1 in ((0, 4), (4, KP)):
                    for s in range(8):
                        nc.scalar.dma_start(
                            tmp16[:, k0:k1, s, :],
                            idx_all[16 * s:16 * (s + 1), k0:k1, :],
                        )
                    nc.vector.tensor_copy(
                        out=idx_wr[0:16, k0:k1, :].rearrange(
                            "p k (j s) -> p k j s", s=8
                        ),
                        in_=tmp16[:, k0:k1].rearrange("p k s j -> p k j s"),
                    )
                    for r in range(1, 8):
                        nc.scalar.dma_start(
                            idx_wr[16 * r:16 * (r + 1), k0:k1, :],
                            idx_wr[0:16, k0:k1, :],
                        )

            # ---------------- per-kp corner weights (overlaps gathers) --
            wpair = mpool.tile([128, KP * 4 * NJ * 2], bff, tag="wpair")
            wpv = wpair[:].rearrange(
                "p (k q j e) -> p k q j e", k=KP, q=4, j=NJ
            )
            for k in range(KP):
                fy = frac[:, 2 * k, :]
                fx = frac[:, 2 * k + 1, :]
                # corner weights: q order [w00, w10, w01, w11] = (dy,dx):
                # q0=(0,0) q1=(1,0) q2=(0,1) q3=(1,1)
                w11 = wpool.tile([128, NJ], f32, tag="w11")
                nc.vector.tensor_tensor(out=w11[:], in0=fy, in1=fx, op=OP.mult)
                w10 = wpool.tile([128, NJ], f32, tag="w10")
                nc.vector.tensor_tensor(out=w10[:], in0=fy, in1=w11[:], op=OP.subtract)
                w01 = wpool.tile([128, NJ], f32, tag="w01")
                nc.vector.tensor_tensor(out=w01[:], in0=fx, in1=w11[:], op=OP.subtract)
                omfy = wpool.tile([128, NJ], f32, tag="omfy")
                nc.scalar.activation(
                    out=omfy[:], in_=fy, func=AF.Identity,
                    bias=c1[:], scale=-1.0,
                )
                w00 = wpool.tile([128, NJ], f32, tag="w00")
                nc.vector.tensor_tensor(out=w00[:], in0=omfy[:], in1=w01[:], op=OP.subtract)
                for q, wq in enumerate([w00, w10, w01, w11]):
                    for e in range(2):
                        nc.vector.tensor_copy(
                            out=wpv[:, k, q, :, e], in_=wq[:]
                        )

            # ---------------- main loop ----------------
            # Per chunk: for each kp, gather + weight-mult + dx-add into
            # S_k [128px, j, dy, c]; PE transposes S_k per 128px block and
            # a matmul per (k, sub) accumulates K-blocks into pout[sub].
            loop_pools = tc.tile_pool(name="gath", bufs=8)
            gpool = loop_pools.__enter__()
            mul_cm = tc.tile_pool(name="mul", bufs=2)
            mulpool = mul_cm.__enter__()
            samp_cm = tc.tile_pool(name="samp", bufs=3)
            spool = samp_cm.__enter__()
            stage_cm = tc.tile_pool(name="stage", bufs=4)
            stpool = stage_cm.__enter__()
            gq = 0
            for cch in [cc for _ in range(repeat) for cc in range(NCHUNK)]:
                pouts = [
                    psB.tile([O, 512], mybir.dt.float32, space="PSUM",
                             tag=f"pout{sub}", name=f"pout{sub}")
                    for sub in range(NSUB)
                ]
                for k in range(KP):
                    gt = gpool.tile([128, NJC, 256], bff, tag="g")
                    nc.gpsimd.dma_gather(
                        out_ap=gt[:],
                        in_ap=bass.AP(P2, 0, [[128, NELEM], [1, 256]]),
                        idxs_ap=idx_wr[:, k, cch * (CHUNK // 16):(cch + 1) * (CHUNK // 16)],
                        num_idxs=CHUNK,
                        num_idxs_reg=CHUNK,
                        elem_size=256,
                        elem_step=128,
                        single_packet=False,
                        queue_num=gq % NQ,
                    )
                    gq += 1
                    mt = [
                        mulpool.tile([128, NJC, 64], bff, tag=f"m{q}", name=f"m{q}")
                        for q in range(4)
                    ]
                    for q in range(4):
                        b_ap = bass.AP(
                            wpair.tensor,
                            wpair[:].offset + ((k * 4 + q) * NJ + cch * NJC) * 2,
                            [[KP * 4 * NJ * 2, 128], [2, NJC], [0, 32], [1, 2]],
                        )
                        nc.vector.tensor_tensor(
                            out=mt[q][:].rearrange("p j (a e) -> p j a e", a=32),
                            in0=gt[:, :, q * 64:(q + 1) * 64].rearrange(
                                "p j (a e) -> p j a e", a=32
                            ),
                            in1=b_ap,
                            op=OP.mult,
                        )
                    # dy-major sk so each add writes one contiguous run
                    # (strided dst drops DVE to 1x mode).
                    sk = spool.tile([128, 2, NJC, 64], bff, tag="sk")
                    nc.vector.tensor_tensor(
                        out=sk[:, 0], in0=mt[0][:], in1=mt[2][:], op=OP.add
                    )
                    nc.vector.tensor_tensor(
                        out=sk[:, 1], in0=mt[1][:], in1=mt[3][:], op=OP.add
                    )

                    for sub in range(NSUB):
                        pt = psA.tile([128, 512], bff, space="PSUM", tag="pt")
                        for jj in range(4):
                            j = sub * 4 + jj
                            for dy in range(2):
                                nc.tensor.transpose(
                                    out=pt[dy * 64:(dy + 1) * 64,
                                           jj * 128:(jj + 1) * 128],
                                    in_=sk[:, dy, j, :],
                                    identity=ident_bf[:],
                                )
                        st = stpool.tile([128, 512], bff, tag="st")
                        nc.scalar.copy(out=st[:], in_=pt[:])
                        nc.tensor.matmul(
                            out=pouts[sub][:],
                            lhsT=wT_sb[:, k * O:(k + 1) * O],
                            rhs=st[:],
                            start=(k == 0),
                            stop=(k == KP - 1),
                        )

                for sub in range(NSUB):
                    ob = stpool.tile([O, 512], mybir.dt.float32, tag="ob")
                    nc.scalar.activation(
                        out=ob[:], in_=pouts[sub][:], func=AF.Identity,
                        bias=bias_sb[:], scale=1.0,
                    )
                    nc.sync.dma_start(
                        out.ap()[:, cch * CHUNK + sub * 512: cch * CHUNK + (sub + 1) * 512],
                        ob[:],
                    )
            stage_cm.__exit__(None, None, None)
            samp_cm.__exit__(None, None, None)
            mul_cm.__exit__(None, None, None)
            loop_pools.__exit__(None, None, None)

    nc.compile()
    return nc


def _get_program():
    if "nc" not in _CACHE:
        _CACHE["nc"] = _build_program()
    return _CACHE["nc"]


def kernel(x, offset, weight, bias):
    import os
    from concourse.bass_utils import run_bass_kernel_spmd

    x = np.asarray(x, dtype=np.float32)
    offset = np.asarray(offset, dtype=np.float32)
    weight = np.asarray(weight, dtype=np.float32)
    bias = np.asarray(bias, dtype=np.float32)
    B = x.shape[0]
    assert B == N_CORES

    w3 = weight.reshape(O, C, KP)
    # K-block k rows = (dy, c), same conv weights for both dy (the dy-sum
    # of bilinear corners is absorbed into the contraction).
    wTn = np.zeros((KP, 128, O), dtype=bf16)
    for k in range(KP):
        wk = w3[:, :, k].T.astype(bf16)          # [C, O]
        wTn[k, 0:64, :] = wk
        wTn[k, 64:128, :] = wk
    bias_n = bias.reshape(O, 1).astype(np.float32)

    in_maps = []
    for b in range(B):
        in_maps.append({
            "xin": x[b].reshape(C, PX),
            "offin": offset[b].reshape(2 * KP, PX),
            "wT": wTn,
            "bin": bias_n,
        })

    nc = _get_program()
    trace = os.environ.get("DC_TRACE") == "1"
    res = run_bass_kernel_spmd(
        nc, in_maps, list(range(N_CORES)),
        trace=trace, tmpdir=os.environ.get("DC_TRACE_DIR"),
    )
    if res.exec_time_ns is not None:
        _CACHE["exec_time_ns"] = res.exec_time_ns
    outs = [res.results[b]["out"].reshape(O, H, W) for b in range(B)]
    return np.stack(outs, axis=0).astype(np.float32)


# revision 31
# speedup vs baseline: 1.4920x; 1.4920x over previous
"""Deformable Conv2d (B=8, C=O=64, H=W=128, K=3) on 8 Trainium2 NeuronCores.

Data-parallel over batch: core b handles batch b.

Per-core algorithm (all on device):
  1. Pad x by 2 (zeros) -> x_pad2 [132,132] so every clamped bilinear corner
     reads an exact zero (reference's out-of-bounds corners contribute 0).
  2. Build P2 in DRAM: token t=(y*132+x) holds [x_pad2[y,x,:64c], x_pad2[y+1,x,:64c]]
     in bf16 (256B). A 512B gather elem starting at token t covers the full
     2x2 corner patch (x and x+1 columns, y and y+1 rows) via elem_step=128.
  3. Pixel-major index/weight math on DVE: y0=floor via round-to-nearest
     magic (the -0.5 is baked into the bx/by base tables), clamp to
     [-1,129], idx=(y0+1)*132+(x0+1); 4 bilinear corner weights.
  4. dma_gather (px-major): multiply by corner weights (pair-packed
     broadcast b operand, bf16 2x mode); 2 adds reduce the dx corner pairs
     -> S_k [128px, j, dy, c]. The dy-sum is absorbed into the conv matmul
     by duplicating weight rows (K-block k = (dy,c), W rows repeated for
     both dy). Gathers are spread round-robin over 4 SWDGE queues so 4 Q7
     core pairs generate descriptors in parallel.
  5. Per kp: PE-transpose S_k to [(dy,c), px]; 9 accumulating matmuls
     -> out[o, px]; ACT bias.
"""

import numpy as np
import ml_dtypes

C = 64
O = 64
H = 128
W = 128
KP = 9
PX = H * W                    # 16384
W2 = 132                      # padded-by-2 width
NTOK = W2 * W2                # 17424
NB = 138                      # x_sb col blocks (138*128 = 17664 >= NTOK+132+...)
NELEM = NTOK - 1              # gather index bound (reads tokens idx, idx+1)
MAGIC = 12582912.0            # 3 * 2**22, f32 round-to-nearest magic
CHUNK = 2048                  # px per gather chunk
NCHUNK = PX // CHUNK          # 8
NJC = CHUNK // 128            # 16 j-blocks per chunk
NJ = PX // 128                # 128 j-blocks total
NSUB = CHUNK // 512           # 4 psum output tiles per chunk
N_CORES = 8
NQ = 4                        # SWDGE queues (Q7 core pairs) for gathers

bf16 = ml_dtypes.bfloat16

_CACHE = {}


def _build_program(repeat=1):
    import concourse.bacc as bacc
    import concourse.bass as bass
    import concourse.mybir as mybir
    import concourse.tile as tile
    from concourse import library_config
    from concourse.masks import make_identity

    f32 = mybir.dt.float32
    bff = mybir.dt.bfloat16
    i16 = mybir.dt.int16
    i32 = mybir.dt.int32
    AF = mybir.ActivationFunctionType
    OP = mybir.AluOpType

    nc = bacc.Bacc(
        "TRN2", target_bir_lowering=False, debug=False, num_swdge_queues=NQ,
        dynamic_dma_scratch_size=32768,
    )

    xin = nc.dram_tensor("xin", [C, PX], f32, kind="ExternalInput")
    offin = nc.dram_tensor("offin", [2 * KP, PX], f32, kind="ExternalInput")
    wT = nc.dram_tensor("wT", [KP, 128, O], bff, kind="ExternalInput")
    bin_ = nc.dram_tensor("bin", [O, 1], f32, kind="ExternalInput")
    out = nc.dram_tensor("out", [O, PX], f32, kind="ExternalOutput")
    P2 = nc.dram_tensor("P2", [NTOK * 128], bff)

    with tile.TileContext(nc) as tc:
        with (
            tc.tile_pool(name="const", bufs=1) as cpool,
            tc.tile_pool(name="main", bufs=1) as mpool,
            tc.tile_pool(name="wtmp", bufs=2) as wpool,
            tc.tile_pool(name="psA", bufs=2, space="PSUM") as psA,
            tc.tile_pool(name="psB", bufs=1, space="PSUM") as psB,
        ):
            nc.gpsimd.load_library(library_config.mlp)
            # ---------------- constants ----------------
            ident_bf = cpool.tile([128, 128], bff)
            make_identity(nc, ident_bf[:])
            ident_f = cpool.tile([128, 128], f32)
            make_identity(nc, ident_f[:])

            wT_sb = cpool.tile([128, KP * O], bff)
            nc.sync.dma_start(
                wT_sb[:],
                bass.AP(wT, 0, [[O, 128], [128 * O, KP], [1, O]]),
            )
            bias_sb = cpool.tile([O, 1], f32)
            nc.sync.dma_start(bias_sb[:], bin_.ap())
            c133 = cpool.tile([128, 1], f32, tag="c133")
            nc.vector.memset(c133[:], 133.0)
            c1 = cpool.tile([128, 1], f32, tag="c1")
            nc.vector.memset(c1[:], 1.0)

            # basex (per-partition w) variants: w + kx - 0.5 (the -0.5 turns
            # the magic round-to-nearest into floor; the frac weights add the
            # 0.5 back exactly via scalar_tensor_tensor).
            bx_i = cpool.tile([128, 1], i32)
            nc.gpsimd.iota(bx_i[:], pattern=[[1, 1]], base=0, channel_multiplier=1)
            bx_f = cpool.tile([128, 1], f32)
            nc.vector.tensor_copy(out=bx_f[:], in_=bx_i[:])
            bx = []
            for kxi in range(3):
                t = cpool.tile([128, 1], f32, tag=f"bx{kxi}")
                nc.vector.tensor_scalar(
                    out=t[:], in0=bx_f[:], scalar1=float(kxi) - 0.5, scalar2=None,
                    op0=OP.add,
                )
                bx.append(t)
            # basey (per-column j=h) variants: j + ky - 0.5
            by_i = cpool.tile([128, 128], i32)
            nc.gpsimd.iota(by_i[:], pattern=[[1, 128]], base=0, channel_multiplier=0)
            by_f = cpool.tile([128, 128], f32)
            nc.vector.tensor_copy(out=by_f[:], in_=by_i[:])
            by = []
            for kyi in range(3):
                t = cpool.tile([128, 128], f32, tag=f"by{kyi}")
                nc.vector.tensor_scalar(
                    out=t[:], in0=by_f[:], scalar1=float(kyi) - 0.5, scalar2=None,
                    op0=OP.add,
                )
                by.append(t)

            # ---------------- x -> P2 ----------------
            offp_cm = tc.tile_pool(name="offp", bufs=1)
            opool = offp_cm.__enter__()
            off_px = opool.tile([128, 2 * KP, NJ], f32, tag="offpx")
            # off_f load issued first on sync: it gates the idx/weight chain.
            off_f = opool.tile([2 * KP, PX], f32, tag="offf")
            nc.sync.dma_start(off_f[:], offin.ap())
            with tc.tile_pool(name="prep", bufs=1) as ppool:
                x_sb = ppool.tile([C, NB * 128], bff, tag="xsb")
                # zero only the padding border, not the whole 2.2 MB tile.
                xv = x_sb[:, :NTOK].rearrange("c (a b) -> c a b", a=W2)
                nc.vector.memset(x_sb[:, 0:2 * W2], 0.0)
                nc.vector.memset(x_sb[:, 130 * W2:NB * 128], 0.0)
                nc.vector.memset(xv[:, 2:130, 0:2], 0.0)
                nc.vector.memset(xv[:, 2:130, 130:132], 0.0)
                interior = xv[:, 2:130, 2:130]
                nc.gpsimd.dma_start(
                    out=interior, in_=xin.ap().rearrange("c (h w) -> c h w", h=H)
                )
                x_px = ppool.tile([128, NB, C], bff, tag="xpx")
                nc.sync.dma_start_transpose(x_px[:], x_sb[:])

                # P2 token t first half <- x_tok(t):
                nc.sync.dma_start(
                    bass.AP(P2, 0, [[128, 128], [128 * 128, 136], [1, 64]]),
                    x_px[:, 0:136, :],
                )
                nc.sync.dma_start(
                    bass.AP(P2, 17408 * 128, [[128, 16], [1, 64]]),
                    x_px[0:16, 136, :],
                )
                # P2 token t second half <- x_tok(t+132), written with the
                # -132-token shift on the DRAM side so the SBUF source stays
                # 128-partition aligned (fast descriptors):
                #   t = p + 128b - 132 for src block b>=2  -> t >= 124
                nc.sync.dma_start(
                    bass.AP(P2, 64 + 124 * 128, [[128, 128], [128 * 128, 135], [1, 64]]),
                    x_px[:, 2:137, :],
                )
                #   src block b=1, p>=4  -> t = 0..123
                nc.sync.dma_start(
                    bass.AP(P2, 64, [[128, 124], [1, 64]]),
                    x_px[4:128, 1, :],
                )

                # ---------------- offsets -> px-major ----------------
                for g in range(8):
                    ps = psA.tile([128, 16 * 18], f32, tag="offps")
                    for jj in range(16):
                        j = g * 16 + jj
                        nc.tensor.transpose(
                            out=ps[:, jj * 18:(jj + 1) * 18],
                            in_=off_f[:, j * 128:(j + 1) * 128],
                            identity=ident_f[0:18, 0:18],
                        )
                    nc.vector.tensor_copy(
                        out=off_px[:, :, g * 16:(g + 1) * 16],
                        in_=ps[:].rearrange("p (a b) -> p b a", a=16),
                    )

            # ---------------- per-kp index math ----------------
            idx_all = mpool.tile([128, KP, NJ], i16, tag="idxall")
            frac = mpool.tile([128, 2 * KP, NJ], f32, tag="frac")
            for k in range(KP):
                kyi, kxi = k // 3, k % 3
                oy = off_px[:, 2 * k, :]
                ox = off_px[:, 2 * k + 1, :]
                zy = wpool.tile([128, NJ], f32, tag="zy")
                nc.vector.tensor_tensor(out=zy[:], in0=oy, in1=by[kyi][:], op=OP.add)
                y0 = wpool.tile([128, NJ], f32, tag="y0")
                nc.vector.tensor_scalar(
                    out=y0[:], in0=zy[:], scalar1=MAGIC, scalar2=MAGIC,
                    op0=OP.add, op1=OP.subtract,
                )
                fy = frac[:, 2 * k, :]
                nc.vector.scalar_tensor_tensor(
                    out=fy, in0=zy[:], scalar=0.5, in1=y0[:],
                    op0=OP.add, op1=OP.subtract,
                )
                nc.vector.tensor_scalar(
                    out=y0[:], in0=y0[:], scalar1=-1.0, scalar2=129.0,
                    op0=OP.max, op1=OP.min,
                )
                iy = wpool.tile([128, NJ], f32, tag="iy")
                nc.scalar.activation(
                    out=iy[:], in_=y0[:], func=AF.Identity,
                    bias=c133[:], scale=132.0,
                )

                zx = wpool.tile([128, NJ], f32, tag="zx")
                nc.scalar.activation(
                    out=zx[:], in_=ox, func=AF.Identity,
                    bias=bx[kxi][:], scale=1.0,
                )
                x0 = wpool.tile([128, NJ], f32, tag="x0")
                nc.vector.tensor_scalar(
                    out=x0[:], in0=zx[:], scalar1=MAGIC, scalar2=MAGIC,
                    op0=OP.add, op1=OP.subtract,
                )
                fx = frac[:, 2 * k + 1, :]
                nc.vector.scalar_tensor_tensor(
                    out=fx, in0=zx[:], scalar=0.5, in1=x0[:],
                    op0=OP.add, op1=OP.subtract,
                )
                nc.vector.tensor_scalar(
                    out=x0[:], in0=x0[:], scalar1=-1.0, scalar2=129.0,
                    op0=OP.max, op1=OP.min,
                )
                idxf = wpool.tile([128, NJ], f32, tag="idxf")
                nc.vector.tensor_tensor(out=idxf[:], in0=iy[:], in1=x0[:], op=OP.add)
                nc.vector.tensor_copy(out=idx_all[:, k, :], in_=idxf[:])
            offp_cm.__exit__(None, None, None)

            # ---------------- idx wrap to gather layout ----------------
            # Want: idx_wr[r, k, 8j+s] = idx_all[16s+r, k, j]  (r<16),
            # then replicated to partition groups 16..127. Done in two
            # k-halves so the first gathers start before the whole wrap
            # is finished. Fold/replica DMAs go on the scalar HWDGE queue.
            idx_wr = mpool.tile([128, KP, PX // 16], i16, tag="idxwr")
            with tc.tile_pool(name="wrap", bufs=1) as wrpool:
                tmp16 = wrpool.tile([16, KP, 8, NJ], i16, tag="idxtmp")
                for k0, k1 in ((0, 4), (4, KP)):
                    for s in range(8):
                        nc.scalar.dma_start(
                            tmp16[:, k0:k1, s, :],
                            idx_all[16 * s:16 * (s + 1), k0:k1, :],
                        )
                    nc.vector.tensor_copy(
                        out=idx_wr[0:16, k0:k1, :].rearrange(
                            "p k (j s) -> p k j s", s=8
                        ),
                        in_=tmp16[:, k0:k1].rearrange("p k s j -> p k j s"),
                    )
                    for r in range(1, 8):
                        nc.scalar.dma_start(
                            idx_wr[16 * r:16 * (r + 1), k0:k1, :],
                            idx_wr[0:16, k0:k1, :],
                        )

            # ---------------- per-kp corner weights (overlaps gathers) --
            wpair = mpool.tile([128, KP * 4 * NJ * 2], bff, tag="wpair")
            wpv = wpair[:].rearrange(
                "p (k q j e) -> p k q j e", k=KP, q=4, j=NJ
            )
            for k in range(KP):
                fy = frac[:, 2 * k, :]
                fx = frac[:, 2 * k + 1, :]
                # corner weights: q order [w00, w10, w01, w11] = (dy,dx):
                # q0=(0,0) q1=(1,0) q2=(0,1) q3=(1,1)
                w11 = wpool.tile([128, NJ], f32, tag="w11")
                nc.vector.tensor_tensor(out=w11[:], in0=fy, in1=fx, op=OP.mult)
                w10 = wpool.tile([128, NJ], f32, tag="w10")
                nc.vector.tensor_tensor(out=w10[:], in0=fy, in1=w11[:], op=OP.subtract)
                w01 = wpool.tile([128, NJ], f32, tag="w01")
                nc.vector.tensor_tensor(out=w01[:], in0=fx, in1=w11[:], op=OP.subtract)
                omfy = wpool.tile([128, NJ], f32, tag="omfy")
                nc.scalar.activation(
                    out=omfy[:], in_=fy, func=AF.Identity,
                    bias=c1[:], scale=-1.0,
                )
                w00 = wpool.tile([128, NJ], f32, tag="w00")
                nc.vector.tensor_tensor(out=w00[:], in0=omfy[:], in1=w01[:], op=OP.subtract)
                for q, wq in enumerate([w00, w10, w01, w11]):
                    for e in range(2):
                        nc.vector.tensor_copy(
                            out=wpv[:, k, q, :, e], in_=wq[:]
                        )

            # ---------------- main loop ----------------
            # Per chunk: for each kp, gather + weight-mult + dx-add into
            # S_k [128px, j, dy, c]; PE transposes S_k per 128px block and
            # a matmul per (k, sub) accumulates K-blocks into pout[sub].
            loop_pools = tc.tile_pool(name="gath", bufs=8)
            gpool = loop_pools.__enter__()
            mul_cm = tc.tile_pool(name="mul", bufs=2)
            mulpool = mul_cm.__enter__()
            samp_cm = tc.tile_pool(name="samp", bufs=3)
            spool = samp_cm.__enter__()
            stage_cm = tc.tile_pool(name="stage", bufs=4)
            stpool = stage_cm.__enter__()
            gq = 0
            for cch in [cc for _ in range(repeat) for cc in range(NCHUNK)]:
                pouts = [
                    psB.tile([O, 512], mybir.dt.float32, space="PSUM",
                             tag=f"pout{sub}", name=f"pout{sub}")
                    for sub in range(NSUB)
                ]
                for k in range(KP):
                    gt = gpool.tile([128, NJC, 256], bff, tag="g")
                    nc.gpsimd.dma_gather(
                        out_ap=gt[:],
                        in_ap=bass.AP(P2, 0, [[128, NELEM], [1, 256]]),
                        idxs_ap=idx_wr[:, k, cch * (CHUNK // 16):(cch + 1) * (CHUNK // 16)],
                        num_idxs=CHUNK,
                        num_idxs_reg=CHUNK,
                        elem_size=256,
                        elem_step=128,
                        single_packet=False,
                        queue_num=gq % NQ,
                    )
                    gq += 1
                    mt = [
                        mulpool.tile([128, NJC, 64], bff, tag=f"m{q}", name=f"m{q}")
                        for q in range(4)
                    ]
                    for q in range(4):
                        b_ap = bass.AP(
                            wpair.tensor,
                            wpair[:].offset + ((k * 4 + q) * NJ + cch * NJC) * 2,
                            [[KP * 4 * NJ * 2, 128], [2, NJC], [0, 32], [1, 2]],
                        )
                        nc.vector.tensor_tensor(
                            out=mt[q][:].rearrange("p j (a e) -> p j a e", a=32),
                            in0=gt[:, :, q * 64:(q + 1) * 64].rearrange(
                                "p j (a e) -> p j a e", a=32
                            ),
                            in1=b_ap,
                            op=OP.mult,
                        )
                    sk = spool.tile([128, NJC, 2, 64], bff, tag="sk")
                    nc.vector.tensor_tensor(
                        out=sk[:, :, 0, :], in0=mt[0][:], in1=mt[2][:], op=OP.add
                    )
                    nc.vector.tensor_tensor(
                        out=sk[:, :, 1, :], in0=mt[1][:], in1=mt[3][:], op=OP.add
                    )

                    for sub in range(NSUB):
                        pt = psA.tile([128, 512], bff, space="PSUM", tag="pt")
                        for jj in range(4):
                            j = sub * 4 + jj
                            nc.tensor.transpose(
                                out=pt[:, jj * 128:(jj + 1) * 128],
                                in_=sk[:, j, :, :].rearrange("p a b -> p (a b)"),
                                identity=ident_bf[:],
                            )
                        st = stpool.tile([128, 512], bff, tag="st")
                        nc.scalar.copy(out=st[:], in_=pt[:])
                        nc.tensor.matmul(
                            out=pouts[sub][:],
                            lhsT=wT_sb[:, k * O:(k + 1) * O],
                            rhs=st[:],
                            start=(k == 0),
                            stop=(k == KP - 1),
                        )

                for sub in range(NSUB):
                    ob = stpool.tile([O, 512], mybir.dt.float32, tag="ob")
                    nc.scalar.activation(
                        out=ob[:], in_=pouts[sub][:], func=AF.Identity,
                        bias=bias_sb[:], scale=1.0,
                    )
                    nc.sync.dma_start(
                        out.ap()[:, cch * CHUNK + sub * 512: cch * CHUNK + (sub + 1) * 512],
                        ob[:],
                    )
            stage_cm.__exit__(None, None, None)
            samp_cm.__exit__(None, None, None)
            mul_cm.__exit__(None, None, None)
            loop_pools.__exit__(None, None, None)

    nc.compile()
    return nc


def _get_program():
    if "nc" not in _CACHE:
        _CACHE["nc"] = _build_program()
    return _CACHE["nc"]


def kernel(x, offset, weight, bias):
    import os
    from concourse.bass_utils import run_bass_kernel_spmd

    x = np.asarray(x, dtype=np.float32)
    offset = np.asarray(offset, dtype=np.float32)
    weight = np.asarray(weight, dtype=np.float32)
    bias = np.asarray(bias, dtype=np.float32)
    B = x.shape[0]
    assert B == N_CORES

    w3 = weight.reshape(O, C, KP)
    # K-block k rows = (dy, c), same conv weights for both dy (the dy-sum
    # of bilinear corners is absorbed into the contraction).
    wTn = np.zeros((KP, 128, O), dtype=bf16)
    for k in range(KP):
        wk = w3[:, :, k].T.astype(bf16)          # [C, O]
        wTn[k, 0:64, :] = wk
        wTn[k, 64:128, :] = wk
    bias_n = bias.reshape(O, 1).astype(np.float32)

    in_maps = []
    for b in range(B):
        in_maps.append({
            "xin": x[b].reshape(C, PX),
            "offin": offset[b].reshape(2 * KP, PX),
            "wT": wTn,
            "bin": bias_n,
        })

    nc = _get_program()
    trace = os.environ.get("DC_TRACE") == "1"
    res = run_bass_kernel_spmd(
        nc, in_maps, list(range(N_CORES)),
        trace=trace, tmpdir=os.environ.get("DC_TRACE_DIR"),
    )
    if res.exec_time_ns is not None:
        _CACHE["exec_time_ns"] = res.exec_time_ns
    outs = [res.results[b]["out"].reshape(O, H, W) for b in range(B)]
    return np.stack(outs, axis=0).astype(np.float32)


# revision 32
# speedup vs baseline: 1.6823x; 1.1275x over previous
"""Deformable Conv2d (B=8, C=O=64, H=W=128, K=3) on 8 Trainium2 NeuronCores.

Data-parallel over batch: core b handles batch b.

Per-core algorithm (all on device):
  1. Pad x by 2 (zeros) -> x_pad2 [132,132] so every clamped bilinear corner
     reads an exact zero (reference's out-of-bounds corners contribute 0).
  2. Build P2 in DRAM: token t=(y*132+x) holds [x_pad2[y,x,:64c], x_pad2[y+1,x,:64c]]
     in bf16 (256B). A 512B gather elem starting at token t covers the full
     2x2 corner patch (x and x+1 columns, y and y+1 rows) via elem_step=128.
  3. Pixel-major index/weight math on DVE: y0=floor via round-to-nearest
     magic (the -0.5 is baked into the bx/by base tables), clamp to
     [-1,129], idx=(y0+1)*132+(x0+1); 4 bilinear corner weights.
  4. dma_gather (px-major): multiply by corner weights (pair-packed
     broadcast b operand, bf16 2x mode); 2 adds reduce the dx corner pairs
     -> S_k [128px, j, dy, c]. The dy-sum is absorbed into the conv matmul
     by duplicating weight rows (K-block k = (dy,c), W rows repeated for
     both dy). Gathers are spread round-robin over 4 SWDGE queues so 4 Q7
     core pairs generate descriptors in parallel.
  5. Per kp: PE-transpose S_k to [(dy,c), px]; 9 accumulating matmuls
     -> out[o, px]; ACT bias.
"""

import numpy as np
import ml_dtypes

C = 64
O = 64
H = 128
W = 128
KP = 9
PX = H * W                    # 16384
W2 = 132                      # padded-by-2 width
NTOK = W2 * W2                # 17424
NB = 138                      # x_sb col blocks (138*128 = 17664 >= NTOK+132+...)
NELEM = NTOK - 1              # gather index bound (reads tokens idx, idx+1)
MAGIC = 12582912.0            # 3 * 2**22, f32 round-to-nearest magic
CHUNK = 2048                  # px per gather chunk
NCHUNK = PX // CHUNK          # 8
NJC = CHUNK // 128            # 16 j-blocks per chunk
NJ = PX // 128                # 128 j-blocks total
NSUB = CHUNK // 512           # 4 psum output tiles per chunk
N_CORES = 8
NQ = 4                        # SWDGE queues (Q7 core pairs) for gathers

bf16 = ml_dtypes.bfloat16

_CACHE = {}


def _build_program(repeat=1):
    import concourse.bacc as bacc
    import concourse.bass as bass
    import concourse.mybir as mybir
    import concourse.tile as tile
    from concourse import library_config
    from concourse.masks import make_identity

    f32 = mybir.dt.float32
    bff = mybir.dt.bfloat16
    i16 = mybir.dt.int16
    i32 = mybir.dt.int32
    AF = mybir.ActivationFunctionType
    OP = mybir.AluOpType

    nc = bacc.Bacc(
        "TRN2", target_bir_lowering=False, debug=False, num_swdge_queues=NQ,
        dynamic_dma_scratch_size=32768,
    )

    xin = nc.dram_tensor("xin", [C, PX], f32, kind="ExternalInput")
    offin = nc.dram_tensor("offin", [2 * KP, PX], f32, kind="ExternalInput")
    wT = nc.dram_tensor("wT", [KP, 128, O], bff, kind="ExternalInput")
    bin_ = nc.dram_tensor("bin", [O, 1], f32, kind="ExternalInput")
    out = nc.dram_tensor("out", [O, PX], f32, kind="ExternalOutput")
    P2 = nc.dram_tensor("P2", [NTOK * 128], bff)

    with tile.TileContext(nc) as tc:
        with (
            tc.tile_pool(name="const", bufs=1) as cpool,
            tc.tile_pool(name="main", bufs=1) as mpool,
            tc.tile_pool(name="wtmp", bufs=2) as wpool,
            tc.tile_pool(name="psA", bufs=2, space="PSUM") as psA,
            tc.tile_pool(name="psB", bufs=1, space="PSUM") as psB,
        ):
            nc.gpsimd.load_library(library_config.mlp)
            # ---------------- constants ----------------
            ident_bf = cpool.tile([128, 128], bff)
            make_identity(nc, ident_bf[:])
            ident_f = cpool.tile([128, 128], f32)
            make_identity(nc, ident_f[:])

            wT_sb = cpool.tile([128, KP * O], bff)
            nc.sync.dma_start(
                wT_sb[:],
                bass.AP(wT, 0, [[O, 128], [128 * O, KP], [1, O]]),
            )
            bias_sb = cpool.tile([O, 1], f32)
            nc.sync.dma_start(bias_sb[:], bin_.ap())
            c133 = cpool.tile([128, 1], f32, tag="c133")
            nc.vector.memset(c133[:], 133.0)
            c1 = cpool.tile([128, 1], f32, tag="c1")
            nc.vector.memset(c1[:], 1.0)

            # basex (per-partition w) variants: w + kx - 0.5 (the -0.5 turns
            # the magic round-to-nearest into floor; the frac weights add the
            # 0.5 back exactly via scalar_tensor_tensor).
            bx_i = cpool.tile([128, 1], i32)
            nc.gpsimd.iota(bx_i[:], pattern=[[1, 1]], base=0, channel_multiplier=1)
            bx_f = cpool.tile([128, 1], f32)
            nc.vector.tensor_copy(out=bx_f[:], in_=bx_i[:])
            bx = []
            for kxi in range(3):
                t = cpool.tile([128, 1], f32, tag=f"bx{kxi}")
                nc.vector.tensor_scalar(
                    out=t[:], in0=bx_f[:], scalar1=float(kxi) - 0.5, scalar2=None,
                    op0=OP.add,
                )
                bx.append(t)
            # basey (per-column j=h) variants: j + ky - 0.5
            by_i = cpool.tile([128, 128], i32)
            nc.gpsimd.iota(by_i[:], pattern=[[1, 128]], base=0, channel_multiplier=0)
            by_f = cpool.tile([128, 128], f32)
            nc.vector.tensor_copy(out=by_f[:], in_=by_i[:])
            by = []
            for kyi in range(3):
                t = cpool.tile([128, 128], f32, tag=f"by{kyi}")
                nc.vector.tensor_scalar(
                    out=t[:], in0=by_f[:], scalar1=float(kyi) - 0.5, scalar2=None,
                    op0=OP.add,
                )
                by.append(t)

            # ---------------- x -> P2 ----------------
            offp_cm = tc.tile_pool(name="offp", bufs=1)
            opool = offp_cm.__enter__()
            off_px = opool.tile([128, 2 * KP, NJ], f32, tag="offpx")
            # off_f load issued first on sync: it gates the idx/weight chain.
            off_f = opool.tile([2 * KP, PX], f32, tag="offf")
            nc.sync.dma_start(off_f[:], offin.ap())
            with tc.tile_pool(name="prep", bufs=1) as ppool:
                x_sb = ppool.tile([C, NB * 128], bff, tag="xsb")
                # zero only the padding border, not the whole 2.2 MB tile.
                xv = x_sb[:, :NTOK].rearrange("c (a b) -> c a b", a=W2)
                nc.vector.memset(x_sb[:, 0:2 * W2], 0.0)
                nc.vector.memset(x_sb[:, 130 * W2:NB * 128], 0.0)
                nc.vector.memset(xv[:, 2:130, 0:2], 0.0)
                nc.vector.memset(xv[:, 2:130, 130:132], 0.0)
                interior = xv[:, 2:130, 2:130]
                nc.gpsimd.dma_start(
                    out=interior, in_=xin.ap().rearrange("c (h w) -> c h w", h=H)
                )
                x_px = ppool.tile([128, NB, C], bff, tag="xpx")
                nc.sync.dma_start_transpose(x_px[:], x_sb[:])

                # P2 token t first half <- x_tok(t):
                nc.sync.dma_start(
                    bass.AP(P2, 0, [[128, 128], [128 * 128, 136], [1, 64]]),
                    x_px[:, 0:136, :],
                )
                nc.sync.dma_start(
                    bass.AP(P2, 17408 * 128, [[128, 16], [1, 64]]),
                    x_px[0:16, 136, :],
                )
                # P2 token t second half <- x_tok(t+132), written with the
                # -132-token shift on the DRAM side so the SBUF source stays
                # 128-partition aligned (fast descriptors):
                #   t = p + 128b - 132 for src block b>=2  -> t >= 124
                nc.sync.dma_start(
                    bass.AP(P2, 64 + 124 * 128, [[128, 128], [128 * 128, 135], [1, 64]]),
                    x_px[:, 2:137, :],
                )
                #   src block b=1, p>=4  -> t = 0..123
                nc.sync.dma_start(
                    bass.AP(P2, 64, [[128, 124], [1, 64]]),
                    x_px[4:128, 1, :],
                )

                # ---------------- offsets -> px-major ----------------
                for g in range(8):
                    ps = psA.tile([128, 16 * 18], f32, tag="offps")
                    for jj in range(16):
                        j = g * 16 + jj
                        nc.tensor.transpose(
                            out=ps[:, jj * 18:(jj + 1) * 18],
                            in_=off_f[:, j * 128:(j + 1) * 128],
                            identity=ident_f[0:18, 0:18],
                        )
                    nc.vector.tensor_copy(
                        out=off_px[:, :, g * 16:(g + 1) * 16],
                        in_=ps[:].rearrange("p (a b) -> p b a", a=16),
                    )

            # ---------------- per-kp index math ----------------
            idx_all = mpool.tile([128, KP, NJ], i16, tag="idxall")
            frac = mpool.tile([128, 2 * KP, NJ], f32, tag="frac")
            for k in range(KP):
                kyi, kxi = k // 3, k % 3
                oy = off_px[:, 2 * k, :]
                ox = off_px[:, 2 * k + 1, :]
                zy = wpool.tile([128, NJ], f32, tag="zy")
                nc.vector.tensor_tensor(out=zy[:], in0=oy, in1=by[kyi][:], op=OP.add)
                y0 = wpool.tile([128, NJ], f32, tag="y0")
                nc.vector.tensor_scalar(
                    out=y0[:], in0=zy[:], scalar1=MAGIC, scalar2=MAGIC,
                    op0=OP.add, op1=OP.subtract,
                )
                fy = frac[:, 2 * k, :]
                nc.vector.scalar_tensor_tensor(
                    out=fy, in0=zy[:], scalar=0.5, in1=y0[:],
                    op0=OP.add, op1=OP.subtract,
                )
                nc.vector.tensor_scalar(
                    out=y0[:], in0=y0[:], scalar1=-1.0, scalar2=129.0,
                    op0=OP.max, op1=OP.min,
                )
                iy = wpool.tile([128, NJ], f32, tag="iy")
                nc.vector.tensor_scalar(
                    out=iy[:], in0=y0[:], scalar1=132.0, scalar2=133.0,
                    op0=OP.mult, op1=OP.add,
                )

                zx = wpool.tile([128, NJ], f32, tag="zx")
                nc.vector.tensor_scalar(
                    out=zx[:], in0=ox, scalar1=bx[kxi][:], scalar2=None, op0=OP.add
                )
                x0 = wpool.tile([128, NJ], f32, tag="x0")
                nc.vector.tensor_scalar(
                    out=x0[:], in0=zx[:], scalar1=MAGIC, scalar2=MAGIC,
                    op0=OP.add, op1=OP.subtract,
                )
                fx = frac[:, 2 * k + 1, :]
                nc.vector.scalar_tensor_tensor(
                    out=fx, in0=zx[:], scalar=0.5, in1=x0[:],
                    op0=OP.add, op1=OP.subtract,
                )
                nc.vector.tensor_scalar(
                    out=x0[:], in0=x0[:], scalar1=-1.0, scalar2=129.0,
                    op0=OP.max, op1=OP.min,
                )
                idxf = wpool.tile([128, NJ], f32, tag="idxf")
                nc.vector.tensor_tensor(out=idxf[:], in0=iy[:], in1=x0[:], op=OP.add)
                nc.vector.tensor_copy(out=idx_all[:, k, :], in_=idxf[:])
            offp_cm.__exit__(None, None, None)

            # ---------------- idx wrap to gather layout ----------------
            # Want: idx_wr[r, k, 8j+s] = idx_all[16s+r, k, j]  (r<16),
            # then replicated to partition groups 16..127. Done in two
            # k-halves so the first gathers start before the whole wrap
            # is finished. Fold/replica DMAs go on the scalar HWDGE queue.
            idx_wr = mpool.tile([128, KP, PX // 16], i16, tag="idxwr")
            with tc.tile_pool(name="wrap", bufs=1) as wrpool:
                tmp16 = wrpool.tile([16, KP, 8, NJ], i16, tag="idxtmp")
                for k0, k1 in ((0, 4), (4, KP)):
                    for s in range(8):
                        nc.scalar.dma_start(
                            tmp16[:, k0:k1, s, :],
                            idx_all[16 * s:16 * (s + 1), k0:k1, :],
                        )
                    nc.vector.tensor_copy(
                        out=idx_wr[0:16, k0:k1, :].rearrange(
                            "p k (j s) -> p k j s", s=8
                        ),
                        in_=tmp16[:, k0:k1].rearrange("p k s j -> p k j s"),
                    )
                    for r in range(1, 8):
                        nc.scalar.dma_start(
                            idx_wr[16 * r:16 * (r + 1), k0:k1, :],
                            idx_wr[0:16, k0:k1, :],
                        )

            # ---------------- per-kp corner weights (overlaps gathers) --
            wpair = mpool.tile([128, KP * 4 * NJ * 2], bff, tag="wpair")
            wpv = wpair[:].rearrange(
                "p (k q j e) -> p k q j e", k=KP, q=4, j=NJ
            )
            for k in range(KP):
                fy = frac[:, 2 * k, :]
                fx = frac[:, 2 * k + 1, :]
                # corner weights: q order [w00, w10, w01, w11] = (dy,dx):
                # q0=(0,0) q1=(1,0) q2=(0,1) q3=(1,1)
                w11 = wpool.tile([128, NJ], f32, tag="w11")
                nc.vector.tensor_tensor(out=w11[:], in0=fy, in1=fx, op=OP.mult)
                w10 = wpool.tile([128, NJ], f32, tag="w10")
                nc.vector.tensor_tensor(out=w10[:], in0=fy, in1=w11[:], op=OP.subtract)
                w01 = wpool.tile([128, NJ], f32, tag="w01")
                nc.vector.tensor_tensor(out=w01[:], in0=fx, in1=w11[:], op=OP.subtract)
                omfy = wpool.tile([128, NJ], f32, tag="omfy")
                nc.vector.tensor_scalar(
                    out=omfy[:], in0=fy, scalar1=-1.0, scalar2=1.0,
                    op0=OP.mult, op1=OP.add,
                )
                w00 = wpool.tile([128, NJ], f32, tag="w00")
                nc.vector.tensor_tensor(out=w00[:], in0=omfy[:], in1=w01[:], op=OP.subtract)
                for q, wq in enumerate([w00, w10, w01, w11]):
                    for e in range(2):
                        nc.vector.tensor_copy(
                            out=wpv[:, k, q, :, e], in_=wq[:]
                        )

            # ---------------- main loop ----------------
            # Per chunk: for each kp, gather + weight-mult + dx-add into
            # S_k [128px, j, dy, c]; PE transposes S_k per 128px block and
            # a matmul per (k, sub) accumulates K-blocks into pout[sub].
            loop_pools = tc.tile_pool(name="gath", bufs=8)
            gpool = loop_pools.__enter__()
            mul_cm = tc.tile_pool(name="mul", bufs=2)
            mulpool = mul_cm.__enter__()
            samp_cm = tc.tile_pool(name="samp", bufs=3)
            spool = samp_cm.__enter__()
            stage_cm = tc.tile_pool(name="stage", bufs=4)
            stpool = stage_cm.__enter__()
            gq = 0
            for cch in [cc for _ in range(repeat) for cc in range(NCHUNK)]:
                pouts = [
                    psB.tile([O, 512], mybir.dt.float32, space="PSUM",
                             tag=f"pout{sub}", name=f"pout{sub}")
                    for sub in range(NSUB)
                ]
                for k in range(KP):
                    gt = gpool.tile([128, NJC, 256], bff, tag="g")
                    nc.gpsimd.dma_gather(
                        out_ap=gt[:],
                        in_ap=bass.AP(P2, 0, [[128, NELEM], [1, 256]]),
                        idxs_ap=idx_wr[:, k, cch * (CHUNK // 16):(cch + 1) * (CHUNK // 16)],
                        num_idxs=CHUNK,
                        num_idxs_reg=CHUNK,
                        elem_size=256,
                        elem_step=128,
                        single_packet=False,
                        queue_num=gq % NQ,
                    )
                    gq += 1
                    mt = [
                        mulpool.tile([128, NJC, 64], bff, tag=f"m{q}", name=f"m{q}")
                        for q in range(4)
                    ]
                    for q in range(4):
                        b_ap = bass.AP(
                            wpair.tensor,
                            wpair[:].offset + ((k * 4 + q) * NJ + cch * NJC) * 2,
                            [[KP * 4 * NJ * 2, 128], [2, NJC], [0, 32], [1, 2]],
                        )
                        nc.vector.tensor_tensor(
                            out=mt[q][:].rearrange("p j (a e) -> p j a e", a=32),
                            in0=gt[:, :, q * 64:(q + 1) * 64].rearrange(
                                "p j (a e) -> p j a e", a=32
                            ),
                            in1=b_ap,
                            op=OP.mult,
                        )
                    sk = spool.tile([128, NJC, 2, 64], bff, tag="sk")
                    nc.vector.tensor_tensor(
                        out=sk[:, :, 0, :], in0=mt[0][:], in1=mt[2][:], op=OP.add
                    )
                    nc.vector.tensor_tensor(
                        out=sk[:, :, 1, :], in0=mt[1][:], in1=mt[3][:], op=OP.add
                    )

                    for sub in range(NSUB):
                        pt = psA.tile([128, 512], bff, space="PSUM", tag="pt")
                        for jj in range(4):
                            j = sub * 4 + jj
                            nc.tensor.transpose(
                                out=pt[:, jj * 128:(jj + 1) * 128],
                                in_=sk[:, j, :, :].rearrange("p a b -> p (a b)"),
                                identity=ident_bf[:],
                            )
                        st = stpool.tile([128, 512], bff, tag="st")
                        nc.scalar.copy(out=st[:], in_=pt[:])
                        nc.tensor.matmul(
                            out=pouts[sub][:],
                            lhsT=wT_sb[:, k * O:(k + 1) * O],
                            rhs=st[:],
                            start=(k == 0),
                            stop=(k == KP - 1),
                        )

                for sub in range(NSUB):
                    ob = stpool.tile([O, 512], mybir.dt.float32, tag="ob")
                    nc.scalar.activation(
                        out=ob[:], in_=pouts[sub][:], func=AF.Identity,
                        bias=bias_sb[:], scale=1.0,
                    )
                    nc.sync.dma_start(
                        out.ap()[:, cch * CHUNK + sub * 512: cch * CHUNK + (sub + 1) * 512],
                        ob[:],
                    )
            stage_cm.__exit__(None, None, None)
            samp_cm.__exit__(None, None, None)
            mul_cm.__exit__(None, None, None)
            loop_pools.__exit__(None, None, None)

    nc.compile()
    return nc


def _get_program():
    if "nc" not in _CACHE:
        _CACHE["nc"] = _build_program()
    return _CACHE["nc"]


def kernel(x, offset, weight, bias):
    import os
    from concourse.bass_utils import run_bass_kernel_spmd

    x = np.asarray(x, dtype=np.float32)
    offset = np.asarray(offset, dtype=np.float32)
    weight = np.asarray(weight, dtype=np.float32)
    bias = np.asarray(bias, dtype=np.float32)
    B = x.shape[0]
    assert B == N_CORES

    w3 = weight.reshape(O, C, KP)
    # K-block k rows = (dy, c), same conv weights for both dy (the dy-sum
    # of bilinear corners is absorbed into the contraction).
    wTn = np.zeros((KP, 128, O), dtype=bf16)
    for k in range(KP):
        wk = w3[:, :, k].T.astype(bf16)          # [C, O]
        wTn[k, 0:64, :] = wk
        wTn[k, 64:128, :] = wk
    bias_n = bias.reshape(O, 1).astype(np.float32)

    in_maps = []
    for b in range(B):
        in_maps.append({
            "xin": x[b].reshape(C, PX),
            "offin": offset[b].reshape(2 * KP, PX),
            "wT": wTn,
            "bin": bias_n,
        })

    nc = _get_program()
    trace = os.environ.get("DC_TRACE") == "1"
    res = run_bass_kernel_spmd(
        nc, in_maps, list(range(N_CORES)),
        trace=trace, tmpdir=os.environ.get("DC_TRACE_DIR"),
    )
    if res.exec_time_ns is not None:
        _CACHE["exec_time_ns"] = res.exec_time_ns
    outs = [res.results[b]["out"].reshape(O, H, W) for b in range(B)]
    return np.stack(outs, axis=0).astype(np.float32)
